# revision 8
# baseline (speedup 1.0000x reference)
"""Trainium2 Bass kernel for nn_MultiHeadAttention_57251914056150.

Full-input contract: kernel(**inputs) takes the unsharded numpy inputs and
returns the full [B, S, E] output.

Sharding: rows (batch x causal-balanced query chunk pair). 8 cores =
4 batches x 2 chunk patterns. Pattern A owns q-chunks {0,3} of its batch,
pattern B owns {1,2} (chunks of 512 rows); both patterns carry an equal
causal workload. Two SPMD programs are dispatched concurrently on
devices 0-3 and 4-7.

v2 structure (chunk-outer, pair-inner):
- scores^T[kv,q] = Xk (0.125 Wk Wq^T) Xq^T + c ; G^T and c = 0.125 Wk bq
  are host-precomputed per head; bk provably cancels in softmax. The two
  heads of a pair run as row-tiled concurrent matmuls (K=64 each).
- T1 = G Xq^T computed as 4-way tiled matmuls (0,0)/(64,64); the +c bias
  rides the PSUM->SBUF copy as a tensor_scalar_add.
- exp on ACT; causal masking narrowed to the single 128-wide mixed block
  per diagonal tile (one shared [128,128] mask); the fully-masked column
  range is excluded from exp AND from the PV matmul (no zeroing needed).
- U (incl. softmax denominator via xv ones-column) accumulated per head
  in PSUM; normalization folded into the ctx PSUM->SBUF copy
  (ctxT = (Wv^T U) * broadcast(1/den)); ctx matmuls col/row-paired via
  tile_position (0,0)/(64,64).
- output projection per 512-q chunk overlapped with the next chunk's
  attention; bias via tensor_scalar_add on the PSUM drain.
"""

import numpy as np
import ml_dtypes

import jax
from jax.sharding import Mesh, PartitionSpec
from jax.experimental.shard_map import shard_map

import concourse.bass as bass
import concourse.mybir as mybir
import concourse.tile as tile
from concourse import bacc
from contextlib import ExitStack

B, S, E = 4, 2048, 1024
H, HD = 16, 64
R = 1024  # q rows per core
F32 = mybir.dt.float32
F32R = mybir.dt.float32r
BF16 = mybir.dt.bfloat16
BF16_NP = ml_dtypes.bfloat16
EXP = mybir.ActivationFunctionType.Exp

PATTERNS = ((0, 3), (1, 2))  # q-chunk indices (512 rows each) per program


# ---------------------------------------------------------------- device code


def _emit(nc, tc, ctx, aps, pattern, pairs=8):
    const = ctx.enter_context(tc.tile_pool(name="const", bufs=1))
    xq_pool = ctx.enter_context(tc.tile_pool(name="xq", bufs=2))
    xk_pool = ctx.enter_context(tc.tile_pool(name="xk", bufs=8))
    xv_pool = ctx.enter_context(tc.tile_pool(name="xv", bufs=16))
    t1_pool = ctx.enter_context(tc.tile_pool(name="t1", bufs=8))
    pt_pool = ctx.enter_context(tc.tile_pool(name="pt", bufs=6))
    u2_pool = ctx.enter_context(tc.tile_pool(name="u2", bufs=2))
    dn_pool = ctx.enter_context(tc.tile_pool(name="dn", bufs=8))
    rb_pool = ctx.enter_context(tc.tile_pool(name="rb", bufs=4))
    osb_pool = ctx.enter_context(tc.tile_pool(name="osb", bufs=2))
    sc_ps = ctx.enter_context(tc.tile_pool(name="scps", bufs=2, space="PSUM"))
    flex_ps = ctx.enter_context(tc.tile_pool(name="flex", bufs=4, space="PSUM"))

    dma = nc.sync.dma_start

    # ---- constants (small ones first; wp last, needed only at proj time)
    gt2b = const.tile([128, 512], BF16, tag="gt2b")
    dma(gt2b[:, :], aps["gt2b"])
    msk = const.tile([128, 256], BF16, tag="msk")
    dma(msk[:, :], aps["msk"])
    brow = const.tile([128, 8], F32, tag="brow")
    dma(brow[:, :], aps["brow"])
    wv2 = const.tile([128, 512], BF16, tag="wv2")
    dma(wv2[:, :], aps["wv2"])
    bpp = const.tile([128, 8], F32, tag="bpp")
    dma(bpp[:, :], aps["bpp"])
    ctxT = const.tile([128, 8192], BF16, tag="ctxT")
    wp_sb = const.tile([128, 8192], BF16, tag="wp")

    msk3 = msk[:, :].rearrange("p (l j) -> p l j", l=2)

    # ---- resident inputs: all pairs' K/V (both chunks use them)
    xk_t = [xk_pool.tile([128, 2048], BF16, tag="xk", name=f"xk_{p}") for p in range(pairs)]
    xv_t = [xv_pool.tile([128, 16, 65], BF16, tag="xv", name=f"xv_{h}") for h in range(2 * pairs)]
    for p in range(pairs):
        dma(xk_t[p][:, :], aps["xk"][p])
        dma(xv_t[2 * p][:, :, :], aps["xv"][2 * p])
        dma(xv_t[2 * p + 1][:, :, :], aps["xv"][2 * p + 1])
    for ki in range(8):
        dma(wp_sb[:, ki * 1024 : (ki + 1) * 1024], aps["wp"][ki])

    def load_xq(p):
        t = xq_pool.tile([128, 1024], BF16, tag="xq", name=f"xq_{p}")
        dma(t[:, :], aps["xq2"][p])
        return t

    xq_cur = load_xq(0)

    t1_t = [t1_pool.tile([128, 1024], BF16, tag="t1", name=f"t1_{p}") for p in range(pairs)]

    T_of = [4 * (pattern[0] + 1), 4 * (pattern[1] + 1)]

    pending = []

    def emit_pending():
        while pending:
            pending.pop(0)()

    def make_tail(p, ic, u0, u1):
        def tail():
            u2 = u2_pool.tile([128, 512], BF16, tag="u2", name=f"u2_{ic}_{p}")
            nc.vector.tensor_copy(u2[0:64, :], u0[0:64, :])
            nc.vector.tensor_copy(u2[64:128, :], u1[0:64, :])
            # recip/broadcast require partition-base-0, offset-0 operands
            den0 = dn_pool.tile([1, 512], F32, tag="dn", name=f"dn0_{ic}_{p}")
            den1 = dn_pool.tile([1, 512], F32, tag="dn", name=f"dn1_{ic}_{p}")
            nc.vector.tensor_copy(den0[:, :], u0[64:65, :])
            nc.vector.tensor_copy(den1[:, :], u1[64:65, :])
            rc0 = dn_pool.tile([1, 512], F32, tag="dn", name=f"rc0_{ic}_{p}")
            rc1 = dn_pool.tile([1, 512], F32, tag="dn", name=f"rc1_{ic}_{p}")
            nc.vector.reciprocal_approx_fast(out=rc0[:, :], in_=den0[:, :])
            nc.vector.reciprocal_approx_fast(out=rc1[:, :], in_=den1[:, :])
            rb0 = rb_pool.tile([64, 512], F32, tag="rb", name=f"rb0_{ic}_{p}")
            rb1 = rb_pool.tile([64, 512], F32, tag="rb", name=f"rb1_{ic}_{p}")
            nc.gpsimd.partition_broadcast(rb0[:, :], rc0[0:1, :])
            nc.gpsimd.partition_broadcast(rb1[:, :], rc1[0:1, :])
            ps2 = flex_ps.tile([128, 512], F32, tag="flex", name=f"c2_{ic}_{p}")
            nc.tensor.matmul(
                ps2[0:64, :],
                lhsT=wv2[0:64, p * 64 : (p + 1) * 64],
                rhs=u2[0:64, :],
                start=True,
                stop=True,
                tile_position=(0, 0),
            )
            nc.tensor.matmul(
                ps2[64:128, :],
                lhsT=wv2[64:128, p * 64 : (p + 1) * 64],
                rhs=u2[64:128, :],
                start=True,
                stop=True,
                tile_position=(64, 64),
            )
            col = (ic * 8 + p) * 512
            nc.vector.tensor_mul(ctxT[0:64, col : col + 512], ps2[0:64, :], rb0[:, :])
            nc.vector.tensor_mul(ctxT[64:128, col : col + 512], ps2[64:128, :], rb1[:, :])
        return tail

    def emit_proj(ic, ec):
        po = flex_ps.tile([128, 512], F32, tag="flex", name=f"po_{ic}_{ec}")
        for ki in range(8):
            nc.tensor.matmul(
                po[:, :],
                lhsT=wp_sb[:, ki * 1024 + ec * 128 : ki * 1024 + (ec + 1) * 128],
                rhs=ctxT[:, ic * 4096 + ki * 512 : ic * 4096 + (ki + 1) * 512],
                start=(ki == 0),
                stop=(ki == 7),
            )
        osb = osb_pool.tile([128, 512], F32, tag="osb", name=f"osb_{ic}_{ec}")
        nc.vector.tensor_scalar_add(osb[:, :], po[:, :], bpp[:, ec : ec + 1])
        dma(aps["outT"][ec * 128 : (ec + 1) * 128, ic * 512 : (ic + 1) * 512], osb[:, :])

    for ic in range(2):
        T = T_of[ic]
        qo = ic * 512
        for p in range(pairs):
            if ic == 0:
                # T1 for pair p: 4-way tiled matmuls + bias-add drain
                t1ps = sc_ps.tile([128, 1024], F32, tag="sc", name=f"t1ps_{p}")
                for qn in range(2):
                    nc.tensor.matmul(
                        t1ps[0:64, qn * 512 : (qn + 1) * 512],
                        lhsT=gt2b[0:64, p * 64 : (p + 1) * 64],
                        rhs=xq_cur[0:64, qn * 512 : (qn + 1) * 512],
                        start=True,
                        stop=True,
                        tile_position=(0, 0),
                    )
                    nc.tensor.matmul(
                        t1ps[64:128, qn * 512 : (qn + 1) * 512],
                        lhsT=gt2b[64:128, p * 64 : (p + 1) * 64],
                        rhs=xq_cur[64:128, qn * 512 : (qn + 1) * 512],
                        start=True,
                        stop=True,
                        tile_position=(64, 64),
                    )
                nc.vector.tensor_scalar_add(t1_t[p][:, :], t1ps[:, :], brow[:, p : p + 1])
                if p + 1 < pairs:
                    xq_cur = load_xq(p + 1)
            u0 = flex_ps.tile([65, 512], F32, tag="flex", name=f"u_{ic}_{p}_0")
            u1 = flex_ps.tile([65, 512], F32, tag="flex", name=f"u_{ic}_{p}_1")
            for t in range(T):
                if t == 2:
                    emit_pending()
                sc = sc_ps.tile([128, 1024], F32, tag="sc", name=f"sc_{ic}_{p}_{t}")
                nc.tensor.matmul(
                    sc[:, 0:512],
                    lhsT=xk_t[p][0:64, t * 128 : (t + 1) * 128],
                    rhs=t1_t[p][0:64, qo : qo + 512],
                    start=True,
                    stop=True,
                )
                nc.tensor.matmul(
                    sc[:, 512:1024],
                    lhsT=xk_t[p][64:128, t * 128 : (t + 1) * 128],
                    rhs=t1_t[p][64:128, qo : qo + 512],
                    start=True,
                    stop=True,
                )
                pt = pt_pool.tile([128, 1024], BF16, tag="pt", name=f"pt_{ic}_{p}_{t}")
                diag = t >= T - 4
                o = (t - (T - 4)) * 128 if diag else 0
                if o > 0:
                    pt3 = pt[:, :].rearrange("p (l q) -> p l q", l=2)
                    sc3 = sc[:, :].rearrange("p (l q) -> p l q", l=2)
                    nc.scalar.activation(pt3[:, :, o:], sc3[:, :, o:], EXP)
                else:
                    nc.scalar.activation(pt[:, :], sc[:, :], EXP)
                if diag:
                    pt3m = pt[:, :].rearrange("p (l q) -> p l q", l=2)
                    nc.vector.tensor_mul(
                        pt3m[:, :, o : o + 128], pt3m[:, :, o : o + 128], msk3[:, :, :]
                    )
                nc.tensor.matmul(
                    u0[:, o:512],
                    lhsT=xv_t[2 * p][:, t, :],
                    rhs=pt[:, o:512],
                    start=(t == 0),
                    stop=(t == T - 1),
                )
                nc.tensor.matmul(
                    u1[:, o:512],
                    lhsT=xv_t[2 * p + 1][:, t, :],
                    rhs=pt[:, 512 + o : 1024],
                    start=(t == 0),
                    stop=(t == T - 1),
                )
            pending.append(make_tail(p, ic, u0, u1))
            if ic == 1:
                emit_proj(0, p)
    emit_pending()
    for ec in range(8):
        emit_proj(1, ec)


def _build_program(pattern, pairs=8):
    nc = bacc.Bacc("TRN2", target_bir_lowering=False, debug=False)
    aps = {}

    def inp(name, shape, dt):
        aps[name] = nc.dram_tensor(name, shape, dt, kind="ExternalInput").ap()

    inp("xq2", [8, 128, R], BF16)       # per-pair [Xq^T] head-stacked rows
    inp("xk", [8, 128, S], BF16)        # k_enc^T chunks (head pairs)
    inp("xv", [H, 128, 16, 65], BF16)   # (h, kv%128, kv//128, [V dims | ones])
    inp("gt2b", [128, 512], BF16)       # 0.125 * Wq_h Wk_h^T, pair-stacked
    inp("brow", [128, 8], F32)          # 0.125 * Wk_h bq_h, pair-stacked
    inp("wv2", [128, 512], BF16)        # Wv, pair-stacked
    inp("wp", [8, 128, E], BF16)        # Wp e_in chunks
    inp("bpp", [128, 8], F32)           # bp' = bv@Wp + bp, e_out chunks
    inp("msk", [128, 256], BF16)        # mixed-block causal mask, pair-dup
    aps["outT"] = nc.dram_tensor("outT", [E, R], F32, kind="ExternalOutput").ap()

    with tile.TileContext(nc) as tc, ExitStack() as ctx:
        _emit(nc, tc, ctx, aps, pattern, pairs=pairs)
    nc.compile()
    return nc


# ---------------------------------------------------------------- host runner

_EXEC_CACHE = {}


def _get_runner(pidx, devices, pairs=8):
    """Compile (once) and return a jitted shard_map runner on `devices`."""
    key = (pidx, tuple(d.id for d in devices), pairs)
    if key in _EXEC_CACHE:
        return _EXEC_CACHE[key]

    from concourse.bass2jax import (
        _bass_exec_p,
        install_neuronx_cc_hook,
        partition_id_tensor,
    )

    install_neuronx_cc_hook()
    nc = _build_program(PATTERNS[pidx], pairs=pairs)

    partition_name = nc.partition_id_tensor.name if nc.partition_id_tensor else None
    in_names, out_names, out_avals, out_shapes = [], [], [], []
    for alloc in nc.m.functions[0].allocations:
        if not isinstance(alloc, mybir.MemoryLocationSet):
            continue
        name = alloc.memorylocations[0].name
        if alloc.kind == "ExternalInput":
            if name != partition_name:
                in_names.append(name)
        elif alloc.kind == "ExternalOutput":
            out_names.append(name)
            shape = tuple(alloc.tensor_shape)
            dtype = mybir.dt.np(alloc.dtype)
            out_avals.append(jax.core.ShapedArray(shape, dtype))
            out_shapes.append((shape, dtype))
    n_params = len(in_names)
    all_in_names = list(in_names) + out_names
    if partition_name is not None:
        all_in_names.append(partition_name)
    donate = tuple(range(n_params, n_params + len(out_names)))

    def _body(*args):
        operands = list(args)
        if partition_name is not None:
            operands.append(partition_id_tensor())
        outs = _bass_exec_p.bind(
            *operands,
            out_avals=tuple(out_avals),
            in_names=tuple(all_in_names),
            out_names=tuple(out_names),
            lowering_input_output_aliases=(),
            sim_require_finite=True,
            sim_require_nnan=True,
            nc=nc,
        )
        return tuple(outs)

    mesh = Mesh(np.asarray(devices), ("core",))
    n_out = len(out_names)
    sharded = jax.jit(
        shard_map(
            _body,
            mesh=mesh,
            in_specs=(PartitionSpec("core"),) * (n_params + n_out),
            out_specs=(PartitionSpec("core"),) * n_out,
            check_rep=False,
        ),
        donate_argnums=donate,
        keep_unused=True,
    )
    runner = (sharded, in_names, out_names, out_shapes)
    _EXEC_CACHE[key] = runner
    return runner


def _run_program(pidx, devices, in_maps):
    sharded, in_names, out_names, out_shapes = _get_runner(pidx, devices)
    n_cores = len(devices)
    concat_in = [
        np.concatenate([np.asarray(m[name])[None] for m in in_maps], axis=0).reshape(
            n_cores * np.asarray(in_maps[0][name]).shape[0],
            *np.asarray(in_maps[0][name]).shape[1:],
        )
        for name in in_names
    ]
    concat_zeros = [
        np.zeros((n_cores * shape[0], *shape[1:]), dtype) for shape, dtype in out_shapes
    ]
    out_arrs = sharded(*concat_in, *concat_zeros)
    return out_arrs, out_names, out_shapes, n_cores


# ---------------------------------------------------------------- host prep


def _prep_core_inputs(q, k, v, shared, b, pattern):
    """Per-core input dict for batch b with q-chunk pattern `pattern`."""
    c0, c1 = pattern
    rows = np.concatenate(
        [q[b, c0 * 512 : (c0 + 1) * 512], q[b, c1 * 512 : (c1 + 1) * 512]], axis=0
    )  # [R, E]
    rT = np.ascontiguousarray(rows.T)  # [E, R]
    xq2 = rT.reshape(8, 128, R).astype(BF16_NP)

    m = {kk: vv for kk, vv in shared.items() if not isinstance(kk, tuple)}
    m["xq2"] = xq2
    m["xk"] = shared[("xk", b)]
    m["xv"] = shared[("xv", b)]
    return m


def _prep_shared(q, k, v, Wq, bq, Wk, bk, Wv, bv, Wp, bp):
    sh = {}
    # gt2b[i, p*64+j] = 0.125 * (Wq_h @ Wk_h^T)[i, j], heads pair-stacked
    g = 0.125 * np.einsum("hia,hja->hij", Wq, Wk)  # [H, 64, 64]
    gt2b = np.empty((128, 8 * 64), np.float32)
    wv2 = np.empty((128, 8 * 64), np.float32)
    for p in range(8):
        gt2b[0:64, p * 64 : (p + 1) * 64] = g[2 * p]
        gt2b[64:128, p * 64 : (p + 1) * 64] = g[2 * p + 1]
        wv2[0:64, p * 64 : (p + 1) * 64] = Wv[2 * p]
        wv2[64:128, p * 64 : (p + 1) * 64] = Wv[2 * p + 1]
    sh["gt2b"] = gt2b.astype(BF16_NP)
    sh["wv2"] = wv2.astype(BF16_NP)
    c = 0.125 * np.einsum("hja,ha->hj", Wk, bq)  # [H, 64]
    brow = np.empty((128, 8), np.float32)
    for p in range(8):
        brow[0:64, p] = c[2 * p]
        brow[64:128, p] = c[2 * p + 1]
    sh["brow"] = brow
    sh["wp"] = Wp.reshape(8, 128, E).astype(BF16_NP)
    bpp = bv.reshape(-1) @ Wp + bp  # [E]
    sh["bpp"] = np.ascontiguousarray(bpp.reshape(8, 128).T).astype(np.float32)
    pp = np.arange(128)[:, None]
    jj = np.arange(128)[None, :]
    m1 = (pp <= jj).astype(np.float32)
    sh["msk"] = np.concatenate([m1, m1], axis=1).astype(BF16_NP)  # [128, 256]

    for b in range(B):
        sh[("xk", b)] = np.ascontiguousarray(
            k[b].T.reshape(8, 128, S).astype(BF16_NP)
        )
        # xv_aug: [h, kv%128, kv//128, 65]
        xv = np.empty((H, 128, 16, 65), BF16_NP)
        vT = v[b].astype(np.float32)  # [S, E]
        for h in range(H):
            blk = vT[:, h * 64 : (h + 1) * 64].reshape(16, 128, 64)  # [t, p, d]
            xv[h, :, :, :64] = blk.transpose(1, 0, 2).astype(BF16_NP)
        xv[:, :, :, 64] = np.float32(1.0)
        sh[("xv", b)] = xv
    return sh


# ---------------------------------------------------------------- entry point


def _dispatch(inputs):
    q = np.asarray(inputs["q_encodings"], np.float32)
    k = np.asarray(inputs["k_encodings"], np.float32)
    v = np.asarray(inputs["v_encodings"], np.float32)
    sh = _prep_shared(
        q,
        k,
        v,
        np.asarray(inputs["Wq"], np.float32),
        np.asarray(inputs["bq"], np.float32),
        np.asarray(inputs["Wk"], np.float32),
        np.asarray(inputs["bk"], np.float32),
        np.asarray(inputs["Wv"], np.float32),
        np.asarray(inputs["bv"], np.float32),
        np.asarray(inputs["Wp"], np.float32),
        np.asarray(inputs["bp"], np.float32),
    )
    devices = jax.devices()
    assert len(devices) >= 8, f"need 8 cores, have {len(devices)}"
    maps_a = [_prep_core_inputs(q, k, v, sh, b, PATTERNS[0]) for b in range(B)]
    maps_b = [_prep_core_inputs(q, k, v, sh, b, PATTERNS[1]) for b in range(B)]
    res_a = _run_program(0, devices[0:4], maps_a)
    res_b = _run_program(1, devices[4:8], maps_b)
    return res_a, res_b


def _assemble(res_a, res_b):
    out = np.empty((B, S, E), np.float32)
    for pidx, res in ((0, res_a), (1, res_b)):
        out_arrs, out_names, out_shapes, n_cores = res
        idx = out_names.index("outT")
        arr = np.asarray(out_arrs[idx]).reshape(n_cores, E, R)
        c0, c1 = PATTERNS[pidx]
        for b in range(B):
            oT = arr[b]
            out[b, c0 * 512 : (c0 + 1) * 512] = oT[:, 0:512].T
            out[b, c1 * 512 : (c1 + 1) * 512] = oT[:, 512:1024].T
    return out


def kernel(**inputs):
    if not int(np.asarray(inputs.get("mask", 1))):
        raise NotImplementedError("non-causal (mask=0) path not implemented")
    res_a, res_b = _dispatch(inputs)
    return _assemble(res_a, res_b)


def benchmark(inputs, iters=5):
    """Time the two concurrent device dispatches with device-resident inputs."""
    import time
    from jax.sharding import NamedSharding

    kernel(**inputs)  # warm: compile + first run
    q = np.asarray(inputs["q_encodings"], np.float32)
    k = np.asarray(inputs["k_encodings"], np.float32)
    v = np.asarray(inputs["v_encodings"], np.float32)
    sh = _prep_shared(
        q, k, v,
        np.asarray(inputs["Wq"], np.float32), np.asarray(inputs["bq"], np.float32),
        np.asarray(inputs["Wk"], np.float32), np.asarray(inputs["bk"], np.float32),
        np.asarray(inputs["Wv"], np.float32), np.asarray(inputs["bv"], np.float32),
        np.asarray(inputs["Wp"], np.float32), np.asarray(inputs["bp"], np.float32),
    )
    devices = jax.devices()
    staged = []
    for pidx, devs in ((0, devices[0:4]), (1, devices[4:8])):
        maps = [_prep_core_inputs(q, k, v, sh, b, PATTERNS[pidx]) for b in range(B)]
        sharded, in_names, out_names, out_shapes = _get_runner(pidx, devs)
        mesh = Mesh(np.asarray(devs), ("core",))
        nsh = NamedSharding(mesh, PartitionSpec("core"))
        conc = [
            jax.device_put(
                np.concatenate([np.asarray(m[name])[None] for m in maps], 0).reshape(
                    4 * np.asarray(maps[0][name]).shape[0],
                    *np.asarray(maps[0][name]).shape[1:],
                ),
                nsh,
            )
            for name in in_names
        ]
        zero_batches = [
            [
                jax.device_put(np.zeros((4 * s[0], *s[1:]), d), nsh)
                for s, d in out_shapes
            ]
            for _ in range(iters + 1)
        ]
        for z in zero_batches:
            for a in z:
                a.block_until_ready()
        for a in conc:
            a.block_until_ready()
        staged.append((sharded, conc, zero_batches))

    outs = [s(*c, *zb[iters]) for s, c, zb in staged]
    for o in outs:
        for a in o:
            a.block_until_ready()

    times = []
    for i in range(iters):
        t0 = time.perf_counter()
        outs = [s(*c, *zb[i]) for s, c, zb in staged]
        for o in outs:
            for a in o:
                a.block_until_ready()
        times.append(time.perf_counter() - t0)
    return min(times)


# revision 13
# speedup vs baseline: 1.2844x; 1.2844x over previous
"""Trainium2 Bass kernel for nn_MultiHeadAttention_57251914056150.

Full-input contract: kernel(**inputs) takes the unsharded numpy inputs and
returns the full [B, S, E] output.

Sharding: rows (batch x causal-balanced query chunk pair). 8 cores =
4 batches x 2 chunk patterns. Pattern A owns q-chunks {0,3} of its batch,
pattern B owns {1,2} (chunks of 512 rows); both patterns carry an equal
causal workload. Two SPMD programs are dispatched concurrently on
devices 0-3 and 4-7.

v2 structure (chunk-outer, pair-inner):
- scores^T[kv,q] = Xk (0.125 Wk Wq^T) Xq^T + c ; G^T and c = 0.125 Wk bq
  are host-precomputed per head; bk provably cancels in softmax. The two
  heads of a pair run as row-tiled concurrent matmuls (K=64 each).
- T1 = G Xq^T computed as 4-way tiled matmuls (0,0)/(64,64); the +c bias
  rides the PSUM->SBUF copy as a tensor_scalar_add.
- exp on ACT; causal masking narrowed to the single 128-wide mixed block
  per diagonal tile (one shared [128,128] mask); the fully-masked column
  range is excluded from exp AND from the PV matmul (no zeroing needed).
- U (incl. softmax denominator via xv ones-column) accumulated per head
  in PSUM; normalization folded into the ctx PSUM->SBUF copy
  (ctxT = (Wv^T U) * broadcast(1/den)); ctx matmuls col/row-paired via
  tile_position (0,0)/(64,64).
- output projection per 512-q chunk overlapped with the next chunk's
  attention; bias via tensor_scalar_add on the PSUM drain.
"""

import numpy as np
import ml_dtypes

import jax
from jax.sharding import Mesh, PartitionSpec
from jax.experimental.shard_map import shard_map

import concourse.bass as bass
import concourse.mybir as mybir
import concourse.tile as tile
from concourse import bacc
from contextlib import ExitStack

B, S, E = 4, 2048, 1024
H, HD = 16, 64
R = 1024  # q rows per core
F32 = mybir.dt.float32
F32R = mybir.dt.float32r
BF16 = mybir.dt.bfloat16
BF16_NP = ml_dtypes.bfloat16
EXP = mybir.ActivationFunctionType.Exp

PATTERNS = ((0, 3), (1, 2))  # q-chunk indices (512 rows each) per program


# ---------------------------------------------------------------- device code


def _emit(nc, tc, ctx, aps, pattern, pairs=8):
    const = ctx.enter_context(tc.tile_pool(name="const", bufs=1))
    pt_pool = ctx.enter_context(tc.tile_pool(name="pt", bufs=6))
    u2_pool = ctx.enter_context(tc.tile_pool(name="u2", bufs=4))
    dn_pool = ctx.enter_context(tc.tile_pool(name="dn", bufs=4))
    rb_pool = ctx.enter_context(tc.tile_pool(name="rb", bufs=4))
    osb_pool = ctx.enter_context(tc.tile_pool(name="osb", bufs=2))
    sc_ps = ctx.enter_context(tc.tile_pool(name="scps", bufs=2, space="PSUM"))
    flex_ps = ctx.enter_context(tc.tile_pool(name="flex", bufs=4, space="PSUM"))

    dma = nc.sync.dma_start

    # ---- packed constants + single-tile resident inputs; DMA issuance on the
    # sync sequencer is serial (~0.7us each), so order = first-compute order.
    cb = const.tile([128, 1792], BF16, tag="cb")    # gt2b | msk | wv2(padded)
    cf = const.tile([128, 16], F32, tag="cf")       # brow | bpp
    xq_sb = const.tile([128, 8, 1024], BF16, tag="xq")
    xk_sb = const.tile([128, 8, 2048], BF16, tag="xk")
    xv_sb = const.tile([128, 16, 16, 65], BF16, tag="xv")
    ctxT = const.tile([128, 8192], BF16, tag="ctxT")
    wp_sb = const.tile([128, 8192], BF16, tag="wp")

    dma(cb[:, :], aps["cb"])
    dma(cf[:, :], aps["cf"])
    dma(xq_sb[:, 0:1, :], aps["xq2"][0:1].rearrange("k p e -> p k e"))
    dma(xk_sb[:, 0:1, :], aps["xk"][0:1].rearrange("k p e -> p k e"))
    dma(xv_sb[:, 0:2, :, :], aps["xv"][0:2].rearrange("h p t d -> p h t d"))
    dma(xq_sb[:, 1:4, :], aps["xq2"][1:4].rearrange("k p e -> p k e"))
    dma(xk_sb[:, 1:4, :], aps["xk"][1:4].rearrange("k p e -> p k e"))
    dma(xv_sb[:, 2:8, :, :], aps["xv"][2:8].rearrange("h p t d -> p h t d"))
    dma(xq_sb[:, 4:8, :], aps["xq2"][4:8].rearrange("k p e -> p k e"))
    dma(xk_sb[:, 4:8, :], aps["xk"][4:8].rearrange("k p e -> p k e"))
    dma(xv_sb[:, 8:16, :, :], aps["xv"][8:16].rearrange("h p t d -> p h t d"))
    dma(wp_sb[:, :].rearrange("p (k e) -> p k e", k=8), aps["wp"].rearrange("k p e -> p k e"))

    gt2b = cb[:, 0:512]
    msk3 = cb[:, 512:768].rearrange("p (l j) -> p l j", l=2)
    wv2 = cb[:, 768:1792]
    brow = cf[:, 0:8]
    bpp = cf[:, 8:16]

    t1_t = [const.tile([128, 1024], BF16, tag=f"t1_{p}", name=f"t1_{p}") for p in range(pairs)]

    T_of = [4 * (pattern[0] + 1), 4 * (pattern[1] + 1)]

    pending = []

    def emit_pending():
        while pending:
            pending.pop(0)()

    def make_tail(p, ic, u0, u1):
        def tail():
            # u rows: 0 = softmax denominator (ones-first xv), 1:65 = U
            rc0 = dn_pool.tile([1, 512], F32, tag="dn", name=f"rc0_{ic}_{p}")
            rc1 = dn_pool.tile([1, 512], F32, tag="dn", name=f"rc1_{ic}_{p}")
            nc.vector.reciprocal_approx_fast(out=rc0[:, :], in_=u0[0:1, :])
            nc.vector.reciprocal_approx_fast(out=rc1[:, :], in_=u1[0:1, :])
            u2a = u2_pool.tile([65, 512], BF16, tag="u2", name=f"u2a_{ic}_{p}")
            u2b = u2_pool.tile([65, 512], BF16, tag="u2", name=f"u2b_{ic}_{p}")
            nc.vector.tensor_copy(u2a[:, :], u0[:, :])
            nc.vector.tensor_copy(u2b[:, :], u1[:, :])
            rb0 = rb_pool.tile([64, 512], F32, tag="rb", name=f"rb0_{ic}_{p}")
            rb1 = rb_pool.tile([64, 512], F32, tag="rb", name=f"rb1_{ic}_{p}")
            nc.gpsimd.partition_broadcast(rb0[:, :], rc0[0:1, :])
            nc.gpsimd.partition_broadcast(rb1[:, :], rc1[0:1, :])
            ps2 = flex_ps.tile([128, 512], F32, tag="flex", name=f"c2_{ic}_{p}")
            nc.tensor.matmul(
                ps2[0:64, :],
                lhsT=wv2[0:65, p * 64 : (p + 1) * 64],
                rhs=u2a[:, :],
                start=True,
                stop=True,
                tile_position=(0, 0),
            )
            nc.tensor.matmul(
                ps2[64:128, :],
                lhsT=wv2[0:65, 512 + p * 64 : 512 + (p + 1) * 64],
                rhs=u2b[:, :],
                start=True,
                stop=True,
                tile_position=(0, 64),
            )
            col = (ic * 8 + p) * 512
            nc.vector.tensor_mul(ctxT[0:64, col : col + 512], ps2[0:64, :], rb0[:, :])
            nc.vector.tensor_mul(ctxT[64:128, col : col + 512], ps2[64:128, :], rb1[:, :])
        return tail

    proj_state = {}

    def proj_mm(ic, ec, ki):
        if ki == 0:
            proj_state[(ic, ec)] = flex_ps.tile(
                [128, 512], F32, tag="flex", name=f"po_{ic}_{ec}"
            )
        po = proj_state[(ic, ec)]
        nc.tensor.matmul(
            po[:, :],
            lhsT=wp_sb[:, ki * 1024 + ec * 128 : ki * 1024 + (ec + 1) * 128],
            rhs=ctxT[:, ic * 4096 + ki * 512 : ic * 4096 + (ki + 1) * 512],
            start=(ki == 0),
            stop=(ki == 7),
        )

    def proj_drain(ic, ec):
        po = proj_state.pop((ic, ec))
        osb = osb_pool.tile([128, 512], F32, tag="osb", name=f"osb_{ic}_{ec}")
        nc.vector.tensor_scalar_add(osb[:, :], po[:, :], bpp[:, ec : ec + 1])
        dma(aps["outT"][ec * 128 : (ec + 1) * 128, ic * 512 : (ic + 1) * 512], osb[:, :])

    for ic in range(2):
        T = T_of[ic]
        qo = ic * 512
        for p in range(pairs):
            if ic == 0:
                # T1 for pair p: row-tiled matmul pairs + bias-add drain
                t1ps = sc_ps.tile([128, 1024], F32, tag="sc", name=f"t1ps_{p}")
                for qn in range(2):
                    nc.tensor.matmul(
                        t1ps[0:64, qn * 512 : (qn + 1) * 512],
                        lhsT=gt2b[0:64, p * 64 : (p + 1) * 64],
                        rhs=xq_sb[0:64, p, qn * 512 : (qn + 1) * 512],
                        start=True,
                        stop=True,
                        tile_position=(0, 0),
                    )
                    nc.tensor.matmul(
                        t1ps[64:128, qn * 512 : (qn + 1) * 512],
                        lhsT=gt2b[64:128, p * 64 : (p + 1) * 64],
                        rhs=xq_sb[64:128, p, qn * 512 : (qn + 1) * 512],
                        start=True,
                        stop=True,
                        tile_position=(64, 64),
                    )
                nc.vector.tensor_scalar_add(t1_t[p][:, :], t1ps[:, :], brow[:, p : p + 1])
            u0 = flex_ps.tile([65, 512], F32, tag="flex", name=f"u_{ic}_{p}_0")
            u1 = flex_ps.tile([65, 512], F32, tag="flex", name=f"u_{ic}_{p}_1")
            for t in range(T):
                if t == 2:
                    emit_pending()
                sc = sc_ps.tile([128, 1024], F32, tag="sc", name=f"sc_{ic}_{p}_{t}")
                nc.tensor.matmul(
                    sc[:, 0:512],
                    lhsT=xk_sb[0:64, p, t * 128 : (t + 1) * 128],
                    rhs=t1_t[p][0:64, qo : qo + 512],
                    start=True,
                    stop=True,
                )
                nc.tensor.matmul(
                    sc[:, 512:1024],
                    lhsT=xk_sb[64:128, p, t * 128 : (t + 1) * 128],
                    rhs=t1_t[p][64:128, qo : qo + 512],
                    start=True,
                    stop=True,
                )
                pt = pt_pool.tile([128, 1024], BF16, tag="pt", name=f"pt_{ic}_{p}_{t}")
                diag = t >= T - 4
                o = (t - (T - 4)) * 128 if diag else 0
                if o > 0:
                    pt3 = pt[:, :].rearrange("p (l q) -> p l q", l=2)
                    sc3 = sc[:, :].rearrange("p (l q) -> p l q", l=2)
                    nc.scalar.activation(pt3[:, :, o:], sc3[:, :, o:], EXP)
                else:
                    nc.scalar.activation(pt[:, :], sc[:, :], EXP)
                if diag:
                    pt3m = pt[:, :].rearrange("p (l q) -> p l q", l=2)
                    nc.vector.tensor_mul(
                        pt3m[:, :, o : o + 128], pt3m[:, :, o : o + 128], msk3[:, :, :]
                    )
                nc.tensor.matmul(
                    u0[:, o:512],
                    lhsT=xv_sb[:, 2 * p, t, :],
                    rhs=pt[:, o:512],
                    start=(t == 0),
                    stop=(t == T - 1),
                )
                nc.tensor.matmul(
                    u1[:, o:512],
                    lhsT=xv_sb[:, 2 * p + 1, t, :],
                    rhs=pt[:, 512 + o : 1024],
                    start=(t == 0),
                    stop=(t == T - 1),
                )
                if ic == 1 and 2 <= t <= 9:
                    proj_mm(0, p, t - 2)
                if ic == 1 and t == 10:
                    proj_drain(0, p)
            pending.append(make_tail(p, ic, u0, u1))
    emit_pending()
    for ec in range(8):
        for ki in range(8):
            proj_mm(1, ec, ki)
        proj_drain(1, ec)


def _build_program(pattern, pairs=8):
    nc = bacc.Bacc("TRN2", target_bir_lowering=False, debug=False)
    aps = {}

    def inp(name, shape, dt):
        aps[name] = nc.dram_tensor(name, shape, dt, kind="ExternalInput").ap()

    inp("xq2", [8, 128, R], BF16)       # per-pair [Xq^T] head-stacked rows
    inp("xk", [8, 128, S], BF16)        # k_enc^T chunks (head pairs)
    inp("xv", [H, 128, 16, 65], BF16)   # (h, kv%128, kv//128, [ones | V dims])
    inp("cb", [128, 1792], BF16)        # gt2b | msk | wv2 (zero row 0, padded)
    inp("cf", [128, 16], F32)           # brow | bpp
    inp("wp", [8, 128, E], BF16)        # Wp e_in chunks
    aps["outT"] = nc.dram_tensor("outT", [E, R], F32, kind="ExternalOutput").ap()

    with tile.TileContext(nc) as tc, ExitStack() as ctx:
        _emit(nc, tc, ctx, aps, pattern, pairs=pairs)
    nc.compile()
    return nc


# ---------------------------------------------------------------- host runner

_EXEC_CACHE = {}


def _get_runner(pidx, devices, pairs=8):
    """Compile (once) and return a jitted shard_map runner on `devices`."""
    key = (pidx, tuple(d.id for d in devices), pairs)
    if key in _EXEC_CACHE:
        return _EXEC_CACHE[key]

    from concourse.bass2jax import (
        _bass_exec_p,
        install_neuronx_cc_hook,
        partition_id_tensor,
    )

    install_neuronx_cc_hook()
    nc = _build_program(PATTERNS[pidx], pairs=pairs)

    partition_name = nc.partition_id_tensor.name if nc.partition_id_tensor else None
    in_names, out_names, out_avals, out_shapes = [], [], [], []
    for alloc in nc.m.functions[0].allocations:
        if not isinstance(alloc, mybir.MemoryLocationSet):
            continue
        name = alloc.memorylocations[0].name
        if alloc.kind == "ExternalInput":
            if name != partition_name:
                in_names.append(name)
        elif alloc.kind == "ExternalOutput":
            out_names.append(name)
            shape = tuple(alloc.tensor_shape)
            dtype = mybir.dt.np(alloc.dtype)
            out_avals.append(jax.core.ShapedArray(shape, dtype))
            out_shapes.append((shape, dtype))
    n_params = len(in_names)
    all_in_names = list(in_names) + out_names
    if partition_name is not None:
        all_in_names.append(partition_name)
    donate = tuple(range(n_params, n_params + len(out_names)))

    def _body(*args):
        operands = list(args)
        if partition_name is not None:
            operands.append(partition_id_tensor())
        outs = _bass_exec_p.bind(
            *operands,
            out_avals=tuple(out_avals),
            in_names=tuple(all_in_names),
            out_names=tuple(out_names),
            lowering_input_output_aliases=(),
            sim_require_finite=True,
            sim_require_nnan=True,
            nc=nc,
        )
        return tuple(outs)

    mesh = Mesh(np.asarray(devices), ("core",))
    n_out = len(out_names)
    sharded = jax.jit(
        shard_map(
            _body,
            mesh=mesh,
            in_specs=(PartitionSpec("core"),) * (n_params + n_out),
            out_specs=(PartitionSpec("core"),) * n_out,
            check_rep=False,
        ),
        donate_argnums=donate,
        keep_unused=True,
    )
    runner = (sharded, in_names, out_names, out_shapes)
    _EXEC_CACHE[key] = runner
    return runner


def _run_program(pidx, devices, in_maps):
    sharded, in_names, out_names, out_shapes = _get_runner(pidx, devices)
    n_cores = len(devices)
    concat_in = [
        np.concatenate([np.asarray(m[name])[None] for m in in_maps], axis=0).reshape(
            n_cores * np.asarray(in_maps[0][name]).shape[0],
            *np.asarray(in_maps[0][name]).shape[1:],
        )
        for name in in_names
    ]
    concat_zeros = [
        np.zeros((n_cores * shape[0], *shape[1:]), dtype) for shape, dtype in out_shapes
    ]
    out_arrs = sharded(*concat_in, *concat_zeros)
    return out_arrs, out_names, out_shapes, n_cores


# ---------------------------------------------------------------- host prep


def _prep_core_inputs(q, k, v, shared, b, pattern):
    """Per-core input dict for batch b with q-chunk pattern `pattern`."""
    c0, c1 = pattern
    rows = np.concatenate(
        [q[b, c0 * 512 : (c0 + 1) * 512], q[b, c1 * 512 : (c1 + 1) * 512]], axis=0
    )  # [R, E]
    rT = np.ascontiguousarray(rows.T)  # [E, R]
    xq2 = rT.reshape(8, 128, R).astype(BF16_NP)

    m = {kk: vv for kk, vv in shared.items() if not isinstance(kk, tuple)}
    m["xq2"] = xq2
    m["xk"] = shared[("xk", b)]
    m["xv"] = shared[("xv", b)]
    return m


def _prep_shared(q, k, v, Wq, bq, Wk, bk, Wv, bv, Wp, bp):
    sh = {}
    # gt2b[i, p*64+j] = 0.125 * (Wq_h @ Wk_h^T)[i, j], heads pair-stacked
    g = 0.125 * np.einsum("hia,hja->hij", Wq, Wk)  # [H, 64, 64]
    cb = np.zeros((128, 1792), np.float32)
    for p in range(8):
        cb[0:64, p * 64 : (p + 1) * 64] = g[2 * p]
        cb[64:128, p * 64 : (p + 1) * 64] = g[2 * p + 1]
        # wv2 at cols 768:1792, zero row 0 (den row weight), [65, 512|512]
        cb[1:65, 768 + p * 64 : 768 + (p + 1) * 64] = Wv[2 * p]
        cb[1:65, 1280 + p * 64 : 1280 + (p + 1) * 64] = Wv[2 * p + 1]
    pp = np.arange(128)[:, None]
    jj = np.arange(128)[None, :]
    m1 = (pp <= jj).astype(np.float32)
    cb[:, 512:640] = m1
    cb[:, 640:768] = m1
    sh["cb"] = cb.astype(BF16_NP)
    cf = np.zeros((128, 16), np.float32)
    c = 0.125 * np.einsum("hja,ha->hj", Wk, bq)  # [H, 64]
    for p in range(8):
        cf[0:64, p] = c[2 * p]
        cf[64:128, p] = c[2 * p + 1]
    bpp = bv.reshape(-1) @ Wp + bp  # [E]
    cf[:, 8:16] = bpp.reshape(8, 128).T
    sh["cf"] = cf.astype(np.float32)
    sh["wp"] = Wp.reshape(8, 128, E).astype(BF16_NP)

    for b in range(B):
        sh[("xk", b)] = np.ascontiguousarray(
            k[b].T.reshape(8, 128, S).astype(BF16_NP)
        )
        # xv_aug: [h, kv%128, kv//128, [ones | V dims]]
        xv = np.empty((H, 128, 16, 65), BF16_NP)
        vT = v[b].astype(np.float32)  # [S, E]
        for h in range(H):
            blk = vT[:, h * 64 : (h + 1) * 64].reshape(16, 128, 64)  # [t, p, d]
            xv[h, :, :, 1:65] = blk.transpose(1, 0, 2).astype(BF16_NP)
        xv[:, :, :, 0] = np.float32(1.0)
        sh[("xv", b)] = xv
    return sh


# ---------------------------------------------------------------- entry point


def _dispatch(inputs):
    q = np.asarray(inputs["q_encodings"], np.float32)
    k = np.asarray(inputs["k_encodings"], np.float32)
    v = np.asarray(inputs["v_encodings"], np.float32)
    sh = _prep_shared(
        q,
        k,
        v,
        np.asarray(inputs["Wq"], np.float32),
        np.asarray(inputs["bq"], np.float32),
        np.asarray(inputs["Wk"], np.float32),
        np.asarray(inputs["bk"], np.float32),
        np.asarray(inputs["Wv"], np.float32),
        np.asarray(inputs["bv"], np.float32),
        np.asarray(inputs["Wp"], np.float32),
        np.asarray(inputs["bp"], np.float32),
    )
    devices = jax.devices()
    assert len(devices) >= 8, f"need 8 cores, have {len(devices)}"
    maps_a = [_prep_core_inputs(q, k, v, sh, b, PATTERNS[0]) for b in range(B)]
    maps_b = [_prep_core_inputs(q, k, v, sh, b, PATTERNS[1]) for b in range(B)]
    res_a = _run_program(0, devices[0:4], maps_a)
    res_b = _run_program(1, devices[4:8], maps_b)
    return res_a, res_b


def _assemble(res_a, res_b):
    out = np.empty((B, S, E), np.float32)
    for pidx, res in ((0, res_a), (1, res_b)):
        out_arrs, out_names, out_shapes, n_cores = res
        idx = out_names.index("outT")
        arr = np.asarray(out_arrs[idx]).reshape(n_cores, E, R)
        c0, c1 = PATTERNS[pidx]
        for b in range(B):
            oT = arr[b]
            out[b, c0 * 512 : (c0 + 1) * 512] = oT[:, 0:512].T
            out[b, c1 * 512 : (c1 + 1) * 512] = oT[:, 512:1024].T
    return out


def kernel(**inputs):
    if not int(np.asarray(inputs.get("mask", 1))):
        raise NotImplementedError("non-causal (mask=0) path not implemented")
    res_a, res_b = _dispatch(inputs)
    return _assemble(res_a, res_b)


def benchmark(inputs, iters=5):
    """Time the two concurrent device dispatches with device-resident inputs."""
    import time
    from jax.sharding import NamedSharding

    kernel(**inputs)  # warm: compile + first run
    q = np.asarray(inputs["q_encodings"], np.float32)
    k = np.asarray(inputs["k_encodings"], np.float32)
    v = np.asarray(inputs["v_encodings"], np.float32)
    sh = _prep_shared(
        q, k, v,
        np.asarray(inputs["Wq"], np.float32), np.asarray(inputs["bq"], np.float32),
        np.asarray(inputs["Wk"], np.float32), np.asarray(inputs["bk"], np.float32),
        np.asarray(inputs["Wv"], np.float32), np.asarray(inputs["bv"], np.float32),
        np.asarray(inputs["Wp"], np.float32), np.asarray(inputs["bp"], np.float32),
    )
    devices = jax.devices()
    staged = []
    for pidx, devs in ((0, devices[0:4]), (1, devices[4:8])):
        maps = [_prep_core_inputs(q, k, v, sh, b, PATTERNS[pidx]) for b in range(B)]
        sharded, in_names, out_names, out_shapes = _get_runner(pidx, devs)
        mesh = Mesh(np.asarray(devs), ("core",))
        nsh = NamedSharding(mesh, PartitionSpec("core"))
        conc = [
            jax.device_put(
                np.concatenate([np.asarray(m[name])[None] for m in maps], 0).reshape(
                    4 * np.asarray(maps[0][name]).shape[0],
                    *np.asarray(maps[0][name]).shape[1:],
                ),
                nsh,
            )
            for name in in_names
        ]
        zero_batches = [
            [
                jax.device_put(np.zeros((4 * s[0], *s[1:]), d), nsh)
                for s, d in out_shapes
            ]
            for _ in range(iters + 1)
        ]
        for z in zero_batches:
            for a in z:
                a.block_until_ready()
        for a in conc:
            a.block_until_ready()
        staged.append((sharded, conc, zero_batches))

    outs = [s(*c, *zb[iters]) for s, c, zb in staged]
    for o in outs:
        for a in o:
            a.block_until_ready()

    times = []
    for i in range(iters):
        t0 = time.perf_counter()
        outs = [s(*c, *zb[i]) for s, c, zb in staged]
        for o in outs:
            for a in o:
                a.block_until_ready()
        times.append(time.perf_counter() - t0)
    return min(times)
